# revision 28
# baseline (speedup 1.0000x reference)
"""Davies-Bouldin loss kernel for 8 TRN2 NeuronCores (Bass/Tile).

Data-parallel over N: each core streams its shard of `predicted`,
computes per-class scatter sums (sum_x[c,:], sum_vec[c]) via onehot
matmuls on the tensor engine, all-reduces the [64,257] partials across
the 8 cores, and every core computes the identical scalar loss tail.

Per 128-sample sub-tile:
  gather  : psum_g = O^T-onehot @ [cent*ic | ic^2 | cn2]   (PE)
  x2_i    = sum_d x^2          (ACT Square + accum)
  w_i     = cn2_i - 2*ic_i*<cent_ci, x_i>   (DVE tensor_tensor_reduce,
            scale=-2, initial=cn2 gathered column)
  vec_i   = sqrt(x2*ic2 + w)   (ACT Sqrt with AP scale/bias) -> pad col
  scatter : pacc[64,0:257] += O-onehot^T @ [x | vec]        (PE)
"""

import numpy as np

import concourse.bass as bass
import concourse.mybir as mybir
from concourse.bass_utils import run_bass_kernel_spmd
from concourse.tile import TileContext

C = 64
D = 256
N_FULL = 262144
NCORES = 8
MACRO = 512          # samples per macro tile
A = 4                # sub-tiles per macro
ST = 128             # samples per sub-tile
XC = D + 2           # x | vec | pad (f32r matmul needs even free dim)
F32 = mybir.dt.float32
F32R = mybir.dt.float32r
BF16 = mybir.dt.bfloat16
I16 = mybir.dt.int16

AF = mybir.ActivationFunctionType
OP = mybir.AluOpType


def _split_excess_waits(nc, max_waits=1):
    """This walrus build only accepts one sync-wait per instruction;
    hoist excess waits onto prepended NoOps on the same engine."""
    k = 0
    for f in nc.m.functions:
        for b in f.blocks:
            insts = b.instructions
            if not any(
                i.sync_info and i.sync_info.on_wait and len(i.sync_info.on_wait) > max_waits
                for i in insts
            ):
                continue
            out = []
            for inst in insts:
                si = inst.sync_info
                if si and si.on_wait and len(si.on_wait) > max_waits:
                    waits = list(si.on_wait)
                    extra, keep = waits[:-max_waits], waits[-max_waits:]
                    for j in range(0, len(extra), max_waits):
                        chunk = extra[j:j + max_waits]
                        nop = mybir.InstNoOp(name=f"I-splitw-{k}", ins=[], outs=[])
                        k += 1
                        nop.engine = inst.engine
                        nop.sync_info = mybir.SyncInfo(on_wait=chunk, on_update=[])
                        try:
                            nc.register_instruction(nop, overwrite=True)
                        except Exception:
                            pass
                        out.append(nop)
                    inst.sync_info = mybir.SyncInfo(
                        on_wait=keep, on_update=list(si.on_update or [])
                    )
                out.append(inst)
            b.instructions = out
    return k


def build_module(nshard):
    import os
    skip_loop = os.environ.get("K_SKIP_LOOP") == "1"
    skip_tail = os.environ.get("K_SKIP_TAIL") == "1"
    assert nshard % MACRO == 0
    nm = nshard // MACRO
    if skip_loop:
        nm = 0

    nc = bass.Bass("TRN2", target_bir_lowering=False, debug=False, num_devices=NCORES)

    pred = nc.declare_dram_parameter("pred", [nshard, D], F32, isOutput=False)
    t16g = nc.declare_dram_parameter("t16g", [1, nshard], I16, isOutput=False)
    t16p = nc.declare_dram_parameter("t16p", [128, nshard // 128], I16, isOutput=False)
    table = nc.declare_dram_parameter("table", [C, D + 2], F32, isOutput=False)
    wsc = nc.declare_dram_parameter("wsc", [C, C], F32, isOutput=False)
    eyebig = nc.declare_dram_parameter("eyebig", [C, C], F32, isOutput=False)
    iden = nc.declare_dram_parameter("iden", [C, C], F32, isOutput=False)
    onesc = nc.declare_dram_parameter("onesc", [C, 1], F32, isOutput=False)
    onesr = nc.declare_dram_parameter("onesr", [1, C], F32, isOutput=False)
    iotar = nc.declare_dram_parameter("iotar", [128, A * C], I16, isOutput=False)
    iotac = nc.declare_dram_parameter("iotac", [C, MACRO], I16, isOutput=False)
    cent = nc.declare_dram_parameter("cent", [C, D], F32, isOutput=False)
    dist = nc.declare_dram_parameter("dist", [C, 1], F32, isOutput=False)
    icp = nc.declare_dram_parameter("ic", [C, 1], F32, isOutput=False)
    outp = nc.declare_dram_parameter("out", [1, 1], F32, isOutput=True)
    debug = os.environ.get("K_DEBUG") == "1"
    if debug:
        dbg_acc = nc.declare_dram_parameter("dbg_acc", [C, XC], F32, isOutput=True)
        dbg_all = nc.declare_dram_parameter("dbg_all", [C, XC], F32, isOutput=True)
        dbg_xv = nc.declare_dram_parameter("dbg_xv", [128, A * XC], F32, isOutput=True)
        dbg_pg = nc.declare_dram_parameter("dbg_pg", [128, D + 2], F32, isOutput=True)
        dbg_ot = nc.declare_dram_parameter("dbg_ot", [C, MACRO], F32, isOutput=True)
        dbg_oa = nc.declare_dram_parameter("dbg_oa", [128, A * C], F32, isOutput=True)
        dbg_sm = nc.declare_dram_parameter("dbg_sm", [128, 4], F32, isOutput=True)
        dbg_tl = nc.declare_dram_parameter("dbg_tl", [C, 8], F32, isOutput=True)
        dbg_d2f = nc.declare_dram_parameter("dbg_d2f", [C, C], F32, isOutput=True)
        dbg_lnd = nc.declare_dram_parameter("dbg_lnd", [C, C], F32, isOutput=True)
        dbg_rinv = nc.declare_dram_parameter("dbg_rinv", [C, C], F32, isOutput=True)

    cc_in = nc.dram_tensor("cc_in", [C, XC], F32)
    cc_out = nc.dram_tensor("cc_out", [C, XC], F32)

    cc_sem = nc.alloc_semaphore("cc_sem")
    ccd_sem = nc.alloc_semaphore("ccd_sem")

    with TileContext(nc) as tc:
        with (
            tc.tile_pool(name="consts", bufs=1) as cpool,
            tc.tile_pool(name="xin", bufs=4) as xpool,
            tc.tile_pool(name="onehots", bufs=3) as opool,
            tc.tile_pool(name="tbcast", bufs=3) as tbpool,
            tc.tile_pool(name="smalls", bufs=12) as spool,
            tc.tile_pool(name="scratch", bufs=2) as scpool,
            tc.tile_pool(name="psg", bufs=3, space="PSUM") as pgpool,
            tc.tile_pool(name="psacc", bufs=1, space="PSUM") as papool,
            tc.tile_pool(name="pstail", bufs=1, space="PSUM") as ptpool,
            tc.tile_pool(name="tail", bufs=1) as tpool,
        ):
            # ---- constant loads ----
            sb_table = cpool.tile([C, D + 2], F32R, tag="table")
            nc.sync.dma_start(out=sb_table[:], in_=table[:].bitcast(F32R))
            sb_wsc = cpool.tile([C, C], F32, tag="wsc")
            nc.sync.dma_start(out=sb_wsc[:], in_=wsc[:])
            sb_eyebig = cpool.tile([C, C], F32, tag="eyebig")
            nc.sync.dma_start(out=sb_eyebig[:], in_=eyebig[:])
            sb_iden = cpool.tile([C, C], F32, tag="iden")
            nc.sync.dma_start(out=sb_iden[:], in_=iden[:])
            sb_ones = cpool.tile([C, 1], F32, tag="ones")
            nc.sync.dma_start(out=sb_ones[:], in_=onesc[:])
            sb_onesr = cpool.tile([1, C], F32, tag="onesr")
            nc.sync.dma_start(out=sb_onesr[:], in_=onesr[:])
            sb_iotar = cpool.tile([128, A * C], I16, tag="iotar")
            nc.sync.dma_start(out=sb_iotar[:], in_=iotar[:])
            sb_iotac = cpool.tile([C, MACRO], I16, tag="iotac")
            nc.sync.dma_start(out=sb_iotac[:], in_=iotac[:])
            sb_cent = cpool.tile([C, D], F32, tag="cent")
            nc.sync.dma_start(out=sb_cent[:], in_=cent[:])
            sb_dist = cpool.tile([C, 1], F32, tag="dist")
            nc.sync.dma_start(out=sb_dist[:], in_=dist[:])
            sb_ic = cpool.tile([C, 1], F32, tag="ic")
            nc.sync.dma_start(out=sb_ic[:], in_=icp[:])
            sb_tp = cpool.tile([128, nshard // 128], I16, tag="tp")
            nc.sync.dma_start(out=sb_tp[:], in_=t16p[:])

            pacc = papool.tile([C, XC], F32, tag="pacc")

            iotar3 = sb_iotar[:].rearrange("p (a c) -> p a c", c=C)

            # ---- main loop ----
            for m in range(nm):
                xv = xpool.tile([128, A, XC], F32R, tag="xv")
                src = pred[m * MACRO:(m + 1) * MACRO, :].rearrange(
                    "(p a) d -> p a d", p=128
                )
                nc.sync.dma_start(out=xv[:, :, 0:D], in_=src.bitcast(F32R))

                # tb column j = a*128+p holds target[m*MACRO + 4p + a], so the
                # gather lhsT slice [:, a*128:(a+1)*128] matches xv's
                # partition<->sample layout (sample = 4p + a).
                tb = tbpool.tile([C, MACRO], I16, tag="tb")
                nc.sync.dma_start(
                    out=tb[:],
                    in_=t16g[0:1, m * MACRO:(m + 1) * MACRO].partition_broadcast(C),
                )
                ot = opool.tile([C, MACRO], F32R, tag="ot")
                nc.vector.tensor_tensor(
                    out=ot[:], in0=tb[:], in1=sb_iotac[:], op=OP.is_equal
                )
                oa = opool.tile([128, A, C], F32R, tag="oa")
                nc.vector.tensor_tensor(
                    out=oa[:],
                    in0=sb_tp[:, m * A:(m + 1) * A].to_broadcast((128, A, C)),
                    in1=iotar3,
                    op=OP.is_equal,
                )

                for a in range(A):
                    pg = pgpool.tile([128, D + 2], F32, tag="pg")
                    nc.tensor.matmul(
                        pg[:],
                        lhsT=ot[:, a * ST:(a + 1) * ST],
                        rhs=sb_table[:],
                        start=True,
                        stop=True,
                    )
                    x2 = spool.tile([128, 1], F32, tag="x2")
                    sq_scr = scpool.tile([128, D], BF16, tag="sq_scr")
                    nc.scalar.activation(
                        out=sq_scr[:], in_=xv[:, a, 0:D].bitcast(F32), func=AF.Square,
                        accum_out=x2[:],
                    )
                    w = spool.tile([128, 1], F32, tag="w")
                    tt_scr = scpool.tile([128, D], BF16, tag="tt_scr")
                    nc.vector.scalar_tensor_tensor(
                        out=tt_scr[:],
                        in0=pg[:, 0:D],
                        scalar=-2.0,
                        in1=xv[:, a, 0:D].bitcast(F32),
                        op0=OP.mult,
                        op1=OP.mult,
                        accum_out=w[:],
                    )
                    u = spool.tile([128, 1], F32, tag="u")
                    nc.vector.scalar_tensor_tensor(
                        out=u[:], in0=x2[:], scalar=pg[:, D:D + 1], in1=w[:],
                        op0=OP.mult, op1=OP.add,
                    )
                    v2 = spool.tile([128, 1], F32, tag="v2")
                    nc.vector.tensor_tensor(
                        out=v2[:], in0=u[:], in1=pg[:, D + 1:D + 2], op=OP.add,
                    )
                    nc.scalar.activation(
                        out=xv[:, a, D:D + 1], in_=v2[:], func=AF.Sqrt,
                    )
                    nc.tensor.matmul(
                        pacc[:],
                        lhsT=oa[:, a, :],
                        rhs=xv[:, a, 0:XC],
                        start=(m == 0 and a == 0),
                        stop=(m == nm - 1 and a == A - 1),
                    )
                    if debug and m == 0 and a == 0:
                        pg_sb = scpool.tile([128, D + 2], F32, tag="pg_sb")
                        nc.scalar.copy(out=pg_sb[:], in_=pg[:])
                        nc.sync.dma_start(out=dbg_pg[:], in_=pg_sb[:])
                        sm_sb = scpool.tile([128, 4], F32, tag="sm_sb")
                        nc.vector.tensor_copy(out=sm_sb[:, 0:1], in_=x2[:])
                        nc.vector.tensor_copy(out=sm_sb[:, 1:2], in_=w[:])
                        nc.vector.tensor_copy(out=sm_sb[:, 2:3], in_=u[:])
                        nc.vector.tensor_copy(out=sm_sb[:, 3:4], in_=v2[:])
                        nc.sync.dma_start(out=dbg_sm[:], in_=sm_sb[:])

                if debug and m == 0:
                    nc.sync.dma_start(
                        out=dbg_xv[:],
                        in_=xv[:].bitcast(F32).rearrange("p a x -> p (a x)"),
                    )
                    nc.sync.dma_start(out=dbg_ot[:], in_=ot[:].bitcast(F32))
                    nc.sync.dma_start(
                        out=dbg_oa[:],
                        in_=oa[:].bitcast(F32).rearrange("p a c -> p (a c)"),
                    )

            # ---- all-reduce partials across the 8 cores ----
            acc_sb = tpool.tile([C, XC], F32, tag="acc_sb")
            nc.scalar.copy(out=acc_sb[:], in_=pacc[:])
            allsum = tpool.tile([C, XC], F32, tag="allsum")
            with tc.tile_critical():
                nc.sync.dma_start(out=cc_in[:], in_=acc_sb[:]).then_inc(ccd_sem, 16)
                nc.sync.wait_ge(ccd_sem, 16)
                nc.gpsimd.collective_compute(
                    "AllReduce",
                    OP.add,
                    replica_groups=[list(range(NCORES))],
                    ins=[cc_in[:]],
                    outs=[cc_out[:]],
                ).then_inc(cc_sem, 1)
                nc.sync.wait_ge(cc_sem, 1)
                nc.sync.dma_start(out=allsum[:], in_=cc_out[:]).then_inc(ccd_sem, 16)
                nc.sync.wait_ge(ccd_sem, 32)
            if debug:
                nc.sync.dma_start(out=dbg_acc[:], in_=acc_sb[:])
                nc.sync.dma_start(out=dbg_all[:], in_=allsum[:])

            # ---- scalar loss tail (identical on every core) ----
            cn = tpool.tile([C, D], F32, tag="cn")
            nc.vector.scalar_tensor_tensor(
                out=cn[:], in0=allsum[:, 0:D], scalar=sb_ic[:],
                in1=sb_cent[:], op0=OP.mult, op1=OP.add,
            )
            sq = tpool.tile([C, 1], F32, tag="sq")
            sq_scr2 = tpool.tile([C, D], BF16, tag="sq_scr2")
            nc.scalar.activation(
                out=sq_scr2[:], in_=cn[:], func=AF.Square, accum_out=sq[:]
            )
            absr = tpool.tile([C, 1], F32, tag="absr")
            abs_scr = tpool.tile([C, D], BF16, tag="abs_scr")
            nc.scalar.activation(
                out=abs_scr[:], in_=cn[:], func=AF.Abs, accum_out=absr[:]
            )
            # s = sqrt((dist + sum_vec) * ic^2)
            svp = tpool.tile([C, 1], F32, tag="svp")
            nc.vector.tensor_tensor(
                out=svp[:], in0=allsum[:, D:D + 1], in1=sb_dist[:], op=OP.add
            )
            s_sb = tpool.tile([C, 1], F32, tag="s_sb")
            nc.scalar.activation(
                out=s_sb[:], in_=svp[:], func=AF.Sqrt,
                scale=sb_table[:, D:D + 1].bitcast(F32),
            )
            # cn^T (two 128-wide chunks) for CN = cn @ cn^T
            cnt_sb = tpool.tile([128, 128], F32, tag="cnt_sb")
            for h in range(2):
                pt = ptpool.tile([128, C], F32, tag="pt")
                nc.tensor.transpose(
                    pt[:], in_=cn[:, h * 128:(h + 1) * 128], identity=sb_iden[:]
                )
                nc.scalar.copy(out=cnt_sb[:, h * C:(h + 1) * C], in_=pt[:])
            cnp = ptpool.tile([C, C], F32, tag="cnp")
            for h in range(2):
                nc.tensor.matmul(
                    cnp[:],
                    lhsT=cnt_sb[:, h * C:(h + 1) * C],
                    rhs=cnt_sb[:, h * C:(h + 1) * C],
                    start=(h == 0),
                    stop=(h == 1),
                )
            # d2 = sq_i + sq_j - 2*CN + big*I
            d2a = tpool.tile([C, C], F32, tag="d2a")
            nc.vector.scalar_tensor_tensor(
                out=d2a[:], in0=cnp[:], scalar=-2.0, in1=sb_eyebig[:],
                op0=OP.mult, op1=OP.add,
            )
            d2b = tpool.tile([C, C], F32, tag="d2b")
            nc.vector.tensor_scalar(
                out=d2b[:], in0=d2a[:], scalar1=sq[:], scalar2=None, op0=OP.add
            )
            # sq as a row, broadcast down the partitions
            psr = ptpool.tile([1, C], F32, tag="ptsmall")
            nc.tensor.matmul(
                psr[:], lhsT=sq[:], rhs=sb_iden[:],
                start=True, stop=True,
            )
            sqr_sb = tpool.tile([1, C], F32, tag="sqr_sb")
            nc.scalar.copy(out=sqr_sb[:], in_=psr[:])
            sq_rows = ptpool.tile([C, C], F32, tag="prows")
            nc.tensor.matmul(
                sq_rows[:], lhsT=sb_onesr[:], rhs=sqr_sb[:], start=True, stop=True
            )
            d2f = tpool.tile([C, C], F32, tag="d2f")
            nc.vector.tensor_tensor(
                out=d2f[:], in0=d2b[:], in1=sq_rows[:], op=OP.add
            )
            lnd = tpool.tile([C, C], F32, tag="lnd")
            nc.scalar.activation(out=lnd[:], in_=d2f[:], func=AF.Ln)
            rinv = tpool.tile([C, C], F32, tag="rinv")
            nc.scalar.activation(out=rinv[:], in_=lnd[:], func=AF.Exp, scale=-0.5)
            # s as a row, broadcast
            pss = ptpool.tile([1, C], F32, tag="ptsmall")
            nc.tensor.matmul(
                pss[:], lhsT=s_sb[:], rhs=sb_iden[:],
                start=True, stop=True,
            )
            sr_sb = tpool.tile([1, C], F32, tag="sr_sb")
            nc.scalar.copy(out=sr_sb[:], in_=pss[:])
            s_rows = ptpool.tile([C, C], F32, tag="prows")
            nc.tensor.matmul(
                s_rows[:], lhsT=sb_onesr[:], rhs=sr_sb[:], start=True, stop=True
            )
            # term = wsc * (s_i + s_j) / m
            ssum = tpool.tile([C, C], F32, tag="ssum")
            nc.vector.tensor_scalar(
                out=ssum[:], in0=s_rows[:], scalar1=s_sb[:], scalar2=None,
                op0=OP.add,
            )
            numer = tpool.tile([C, C], F32, tag="numer")
            nc.vector.tensor_tensor(
                out=numer[:], in0=ssum[:], in1=sb_wsc[:], op=OP.mult
            )
            term = tpool.tile([C, C], F32, tag="term")
            nc.vector.tensor_tensor(
                out=term[:], in0=numer[:], in1=rinv[:], op=OP.mult
            )
            tsum = tpool.tile([C, 1], F32, tag="tsum")
            nc.vector.tensor_reduce(
                out=tsum[:], in_=term[:], axis=mybir.AxisListType.X, op=OP.add
            )
            total = tpool.tile([C, 1], F32, tag="total")
            nc.vector.scalar_tensor_tensor(
                out=total[:], in0=absr[:], scalar=1e-6, in1=tsum[:],
                op0=OP.mult, op1=OP.add,
            )
            if debug:
                nc.sync.dma_start(out=dbg_d2f[:], in_=d2f[:])
                nc.sync.dma_start(out=dbg_lnd[:], in_=lnd[:])
                nc.sync.dma_start(out=dbg_rinv[:], in_=rinv[:])
                tl_sb = tpool.tile([C, 8], F32, tag="tl_sb")
                nc.vector.tensor_copy(out=tl_sb[:, 0:1], in_=sq[:])
                nc.vector.tensor_copy(out=tl_sb[:, 1:2], in_=absr[:])
                nc.vector.tensor_reduce(
                    out=tl_sb[:, 2:3], in_=ssum[:], axis=mybir.AxisListType.X,
                    op=OP.max,
                )
                nc.vector.tensor_copy(out=tl_sb[:, 3:4], in_=s_sb[:])
                nc.vector.tensor_copy(out=tl_sb[:, 4:5], in_=tsum[:])
                nc.vector.tensor_copy(out=tl_sb[:, 5:6], in_=total[:])
                nc.vector.tensor_reduce(
                    out=tl_sb[:, 6:7], in_=rinv[:], axis=mybir.AxisListType.X,
                    op=OP.max,
                )
                nc.vector.tensor_reduce(
                    out=tl_sb[:, 7:8], in_=term[:], axis=mybir.AxisListType.X,
                    op=OP.max,
                )
                nc.sync.dma_start(out=dbg_tl[:], in_=tl_sb[:])
            pl = ptpool.tile([1, 1], F32, tag="ptsmall")
            nc.tensor.matmul(
                pl[:], lhsT=sb_ones[:], rhs=total[:],
                start=True, stop=True,
            )
            loss_sb = tpool.tile([1, 1], F32, tag="loss_sb")
            nc.scalar.copy(out=loss_sb[:], in_=pl[:])
            nc.sync.dma_start(out=outp[:], in_=loss_sb[:])

    _split_excess_waits(nc)
    return nc


def make_host_inputs(predicted, centroids, distances, count, class_weights, target,
                     nshard):
    """Returns (shared dict, per-core list of dicts)."""
    cent64 = centroids.astype(np.float64)
    cnt64 = count.astype(np.float64)
    ic64 = 1.0 / cnt64                       # [C,1]
    table = np.empty((C, D + 2), np.float32)
    table[:, 0:D] = (cent64 * ic64).astype(np.float32)
    table[:, D] = (ic64 * ic64)[:, 0].astype(np.float32)
    table[:, D + 1] = np.sum(cent64 * cent64, axis=1).astype(np.float32)

    shared = dict(
        table=table,
        wsc=(class_weights.astype(np.float64) * (C - 1) / C).astype(np.float32),
        eyebig=(np.eye(C) * 1e14).astype(np.float32),
        iden=np.eye(C, dtype=np.float32),
        onesc=np.ones((C, 1), np.float32),
        onesr=np.ones((1, C), np.float32),
        iotar=np.tile(np.arange(C, dtype=np.int16), (128, A)),
        iotac=np.repeat(
            np.arange(C, dtype=np.int16)[:, None], MACRO, axis=1
        ),
        cent=np.ascontiguousarray(centroids.astype(np.float32)),
        dist=np.ascontiguousarray(distances.astype(np.float32)),
        ic=ic64.astype(np.float32),
    )

    per_core = []
    for i in range(NCORES):
        lo, hi = i * nshard, (i + 1) * nshard
        tsh = target[lo:hi].astype(np.int16)
        nm = nshard // MACRO
        t16p = (
            tsh.reshape(nm, 128, A).transpose(1, 0, 2).reshape(128, nm * A)
        )
        t16g = tsh.reshape(nm, 128, A).transpose(0, 2, 1).reshape(1, nshard)
        per_core.append(dict(
            pred=np.ascontiguousarray(predicted[lo:hi]),
            t16g=np.ascontiguousarray(t16g),
            t16p=np.ascontiguousarray(t16p),
            **shared,
        ))
    return per_core


_CACHED = {}


def run_spmd(predicted, centroids, distances, count, class_weights, target,
             trace=False, **kw):
    nshard = predicted.shape[0] // NCORES
    if nshard not in _CACHED:
        _CACHED[nshard] = build_module(nshard)
    nc = _CACHED[nshard]
    in_maps = make_host_inputs(
        predicted, centroids, distances, count, class_weights, target, nshard
    )
    return run_bass_kernel_spmd(nc, in_maps, list(range(NCORES)), trace=trace, **kw)


def kernel(predicted, centroids, distances, count, class_weights, target):
    res = run_spmd(predicted, centroids, distances, count, class_weights, target)
    out = res.results[0]["out"]
    return np.asarray(out).reshape(()).astype(np.float32)
